# revision 10
# baseline (speedup 1.0000x reference)
"""Trainium2 Bass kernel for nn_Net_2_78065325572310.

Computes, for x,y [64,2048], W [32768,2048]:
  hx = tanh(BN_train(x@W.T + b)), hy likewise (b cancels in BN, so unused)
  wta = blockwise (4-wide) winner-take-all (keep ==blockmax, zero rest)
  out = cosine(wta(hx), wta(hy)) per row  -> [64]

Sharding: features (W rows) split across 8 cores, 4096 each. Each core
computes per-row partial (dot, ||hx||^2, ||hy||^2) over its features;
host sums partials across cores and finishes the cosine.

Layout on each core:
  - matmul produces h in [row(128=64x|64y) partitions, feat free] via
    lhsT = xyT k-tile [128k, 128col], rhs = W^T strip [128k, 512feat]
  - h tiles are PE-transposed to L layout [block partitions, rows free],
    where host permutation places WTA-block lane l of block (fc,j) at
    feature column l*128+j of chunk fc. BN stats, tanh and WTA are done
    in L layout at full 128-partition width.
"""

import os
import numpy as np

NCORES = 8
BATCH = 64
K = 2048
KT = 16  # k tiles of 128
NFC = 8  # feature chunks per core
FCW = 512  # features per chunk
NL = 4  # WTA block width (lanes)
F_CORE = NFC * FCW  # 4096 features per core
BN_EPS = 1e-5
COS_EPS = 1e-8

# matmul precision mode: "f32" (exact, 4 cyc/row), "f32r" (tf32-ish, 1 cyc/row),
# "bf16x3" (3-pass bf16 hi/lo split, ~fp32 precision, 3 cyc/row)
MODE = os.environ.get("KERNEL_MM_MODE", "bf16x3")

_CACHE = {}
LAST_RESULTS = None


def _split_sync_waits(nc, cap=1):
    """Walrus in this container rejects instructions with >cap sem waits
    ("Too many sync wait commands"); split excess waits onto preceding
    same-engine NoOps."""
    import concourse.mybir as mybir

    n = 0
    for f in nc.m.functions:
        for bb in f.blocks:
            newl = []
            for inst in bb.instructions:
                si = inst.sync_info
                if si is not None and si.on_wait and len(si.on_wait) > cap:
                    waits = list(si.on_wait)
                    keep, extra = waits[:cap], waits[cap:]
                    for k in range(0, len(extra), cap):
                        nop = mybir.InstNoOp(name=f"{inst.name}-wsplit{k}", ins=[], outs=[])
                        nop.engine = inst.engine
                        nop.sync_info = mybir.SyncInfo(
                            on_wait=extra[k : k + cap], on_update=[]
                        )
                        newl.append(nop)
                        n += 1
                    si.on_wait = keep
                newl.append(inst)
            bb.instructions[:] = newl
    return n


def _build_nc(mode):
    import concourse.bass as bass
    import concourse.mybir as mybir
    import concourse.tile as tile
    from contextlib import ExitStack

    F32 = mybir.dt.float32
    BF16 = mybir.dt.bfloat16
    F32R = mybir.dt.float32r
    OP = mybir.AluOpType
    AX = mybir.AxisListType
    AF = mybir.ActivationFunctionType

    nc = bass.Bass(trn_type="TRN2", target_bir_lowering=False, debug=False)

    mm_dt = {"f32": F32, "f32r": F32R, "bf16x3": BF16}[mode]
    if mode == "bf16x3":
        wa_hi = nc.dram_tensor("wa_hi", [NFC, 128, KT * FCW], BF16, kind="ExternalInput")
        wa_lo = nc.dram_tensor("wa_lo", [NFC, 128, KT * FCW], BF16, kind="ExternalInput")
        xyt_hi = nc.dram_tensor("xyt_hi", [128, K], BF16, kind="ExternalInput")
        xyt_lo = nc.dram_tensor("xyt_lo", [128, K], BF16, kind="ExternalInput")
        w_drams = [wa_hi, wa_lo]
    else:
        wa = nc.dram_tensor("wa", [NFC, 128, KT * FCW], mm_dt, kind="ExternalInput")
        xyt = nc.dram_tensor("xyt", [128, K], mm_dt, kind="ExternalInput")
        w_drams = [wa]
    gl_d = nc.dram_tensor("gl", [128, 64], F32, kind="ExternalInput")
    bl_d = nc.dram_tensor("bl", [128, 64], F32, kind="ExternalInput")
    id_d = nc.dram_tensor("ident", [128, 128], F32, kind="ExternalInput")
    out_d = nc.dram_tensor("out", [1, 192], F32, kind="ExternalOutput")

    with tile.TileContext(nc) as tc, ExitStack() as ctx:
        persist = ctx.enter_context(tc.tile_pool(name="persist", bufs=1))
        wpool = ctx.enter_context(tc.tile_pool(name="wpool", bufs=2))
        hpool = ctx.enter_context(tc.tile_pool(name="hpool", bufs=2))
        sqpool = ctx.enter_context(tc.tile_pool(name="sqpool", bufs=2))
        wtapool = ctx.enter_context(tc.tile_pool(name="wtapool", bufs=2))
        prpool = ctx.enter_context(tc.tile_pool(name="prpool", bufs=2))
        ph_pool = ctx.enter_context(tc.tile_pool(name="ph", bufs=2, space="PSUM"))
        pt_pool = ctx.enter_context(tc.tile_pool(name="pt", bufs=4, space="PSUM"))
        pf_pool = ctx.enter_context(tc.tile_pool(name="pf", bufs=1, space="PSUM"))

        # persistent tiles. DMA order matters for startup latency: xy first
        # (gates the first matmul), then the first W chunks are issued inside
        # the fc loop; ident/gl/bl are needed later and follow from there.
        if mode == "bf16x3":
            xy_hi = persist.tile([128, K], BF16, tag="xyhi")
            nc.sync.dma_start(xy_hi[:], xyt_hi.ap()[:])
            xy_lo = persist.tile([128, K], BF16, tag="xylo")
            nc.sync.dma_start(xy_lo[:], xyt_lo.ap()[:])
        else:
            xy_sb = persist.tile([128, K], mm_dt, tag="xy")
            nc.sync.dma_start(xy_sb[:], xyt.ap()[:])
        ident = persist.tile([128, 128], F32, tag="ident")
        gl = persist.tile([128, 64], F32, tag="gl")
        bl = persist.tile([128, 64], F32, tag="bl")

        ones = persist.tile([128, 1], F32, tag="ones")
        nc.vector.memset(ones[:], 1.0)
        magic = persist.tile([128, 64], mybir.dt.uint32, tag="magic")
        nc.vector.memset(magic[:], 0x5F3759DF)

        L_raw = persist.tile([128, NFC * FCW], F32, tag="L_raw")
        L_t = persist.tile([128, NFC * FCW], F32, tag="L_t")
        sums = persist.tile([128, 64], F32, tag="sums")
        sumsq = persist.tile([128, 64], F32, tag="sumsq")
        mean = persist.tile([128, 64], F32, tag="mean")
        e2 = persist.tile([128, 64], F32, tag="e2")
        msq = persist.tile([128, 64], F32, tag="msq")
        veps = persist.tile([128, 64], F32, tag="veps")
        rst = persist.tile([128, 64], F32, tag="rst")
        nt1 = persist.tile([128, 64], F32, tag="nt1")
        nt2 = persist.tile([128, 64], F32, tag="nt2")
        svec = persist.tile([128, 64], F32, tag="svec")
        mts = persist.tile([128, 64], F32, tag="mts")
        tvec = persist.tile([128, 64], F32, tag="tvec")
        acc = persist.tile([128, 192], F32, tag="acc")
        nc.vector.memset(acc[:], 0.0)
        dtmp = persist.tile([128, 192], F32, tag="dtmp")

        NCH = 4  # W strip DMA chunks (separate tiles so matmuls start early)
        KTC = KT // NCH  # k-tiles per chunk

        def emit_matmul_stats(fc):
            # ---- load W strip chunks and matmul into psum h [128 rows, 512f]
            ph = ph_pool.tile([128, FCW], F32, tag="ph")
            CW = KTC * FCW  # free width of one chunk
            if mode == "bf16x3":
                w_hi, w_lo = [], []
                for ch in range(NCH):
                    th = wpool.tile([128, CW], BF16, tag=f"whi{ch}")
                    nc.sync.dma_start(th[:], wa_hi.ap()[fc][:, ch * CW : (ch + 1) * CW])
                    tl = wpool.tile([128, CW], BF16, tag=f"wlo{ch}")
                    nc.sync.dma_start(tl[:], wa_lo.ap()[fc][:, ch * CW : (ch + 1) * CW])
                    w_hi.append(th)
                    w_lo.append(tl)
                n_mm = 3 * KT
                i = 0
                for kt in range(KT):
                    ch, kl = kt // KTC, kt % KTC
                    pairs = [
                        (xy_hi, w_hi[ch]),
                        (xy_lo, w_hi[ch]),
                        (xy_hi, w_lo[ch]),
                    ]
                    for lhs_t, rhs_t in pairs:
                        nc.tensor.matmul(
                            ph[:],
                            lhs_t[:, kt * 128 : (kt + 1) * 128],
                            rhs_t[:, kl * FCW : (kl + 1) * FCW],
                            start=(i == 0),
                            stop=(i == n_mm - 1),
                        )
                        i += 1
            else:
                w_ch = []
                for ch in range(NCH):
                    t = wpool.tile([128, CW], mm_dt, tag=f"w{ch}")
                    nc.sync.dma_start(t[:], w_drams[0].ap()[fc][:, ch * CW : (ch + 1) * CW])
                    w_ch.append(t)
                for kt in range(KT):
                    ch, kl = kt // KTC, kt % KTC
                    nc.tensor.matmul(
                        ph[:],
                        xy_sb[:, kt * 128 : (kt + 1) * 128],
                        w_ch[ch][:, kl * FCW : (kl + 1) * FCW],
                        start=(kt == 0),
                        stop=(kt == KT - 1),
                    )
            if fc == 0:
                nc.sync.dma_start(ident[:], id_d.ap()[:])
            elif fc == 1:
                nc.sync.dma_start(gl[:], gl_d.ap()[:])
                nc.sync.dma_start(bl[:], bl_d.ap()[:])

            # ---- copy h to SBUF (ACT), transpose 128x128 blocks to L layout
            h_sb = hpool.tile([128, FCW], F32, tag="h")
            nc.scalar.copy(h_sb[:], ph[:])
            for l in range(NL):
                pt = pt_pool.tile([128, 128], F32, tag="pt")
                nc.tensor.transpose(
                    pt[:], h_sb[:, l * 128 : (l + 1) * 128], ident[:]
                )
                nc.scalar.copy(L_raw[:, fc * FCW + l * 128 : fc * FCW + (l + 1) * 128], pt[:])

            Lr_fc = L_raw[:, fc * FCW : (fc + 1) * FCW]
            # ---- BN stats: per-feature sums and sum of squares over 64 rows
            S = slice(fc * 8, fc * 8 + 8)
            nc.vector.tensor_reduce(
                sums[:, S],
                Lr_fc.rearrange("p (l s r) -> p l s r", l=NL, s=2),
                axis=AX.X,
                op=OP.add,
            )
            sq = sqpool.tile([128, FCW], F32, tag="sq")
            nc.scalar.square(sq[:], Lr_fc)
            nc.vector.tensor_reduce(
                sumsq[:, S],
                sq[:].rearrange("p (l s r) -> p l s r", l=NL, s=2),
                axis=AX.X,
                op=OP.add,
            )
        def emit_group_math(g):
            # stats math batched over 4 fc chunks (32 cols): mean, var+eps,
            # rstd via fast-inverse-sqrt + 3 Newton steps, s = rstd*gamma,
            # t = beta - mean*s
            S = slice(g * 32, (g + 1) * 32)
            nc.vector.tensor_scalar(mean[:, S], sums[:, S], 1.0 / BATCH, None, op0=OP.mult)
            nc.vector.tensor_scalar(
                e2[:, S], sumsq[:, S], 1.0 / BATCH, BN_EPS, op0=OP.mult, op1=OP.add
            )
            nc.vector.tensor_tensor(msq[:, S], mean[:, S], mean[:, S], op=OP.mult)
            nc.vector.tensor_tensor(veps[:, S], e2[:, S], msq[:, S], op=OP.subtract)
            vu = veps[:, S].bitcast(mybir.dt.uint32)
            ru = rst[:, S].bitcast(mybir.dt.uint32)
            nc.vector.tensor_scalar(ru, vu, 1, None, op0=OP.logical_shift_right)
            nc.vector.tensor_tensor(ru, magic[:, S], ru, op=OP.subtract)
            for _ in range(3):
                nc.vector.tensor_tensor(nt1[:, S], rst[:, S], rst[:, S], op=OP.mult)
                nc.vector.tensor_tensor(nt2[:, S], veps[:, S], nt1[:, S], op=OP.mult)
                nc.vector.tensor_scalar(
                    nt2[:, S], nt2[:, S], -0.5, 1.5, op0=OP.mult, op1=OP.add
                )
                nc.vector.tensor_tensor(rst[:, S], rst[:, S], nt2[:, S], op=OP.mult)
            nc.vector.tensor_tensor(svec[:, S], rst[:, S], gl[:, S], op=OP.mult)
            nc.vector.tensor_tensor(mts[:, S], mean[:, S], svec[:, S], op=OP.mult)
            nc.vector.tensor_tensor(tvec[:, S], bl[:, S], mts[:, S], op=OP.subtract)

        def emit_tanh_wta(fc):
            # ---- normalize + tanh (fused on ACT, per (lane, x|y) column)
            for l in range(NL):
                for s in range(2):
                    c = fc * 8 + l * 2 + s
                    off = fc * FCW + l * 128 + s * 64
                    nc.scalar.activation(
                        L_t[:, off : off + 64],
                        L_raw[:, off : off + 64],
                        AF.Tanh,
                        bias=tvec[:, c : c + 1],
                        scale=svec[:, c : c + 1],
                    )

            # ---- WTA: blockmax over 4 lanes, keep ==max
            Lt_fc = L_t[:, fc * FCW : (fc + 1) * FCW]
            m01 = wtapool.tile([128, 128], F32, tag="m01")
            nc.vector.tensor_tensor(m01[:], Lt_fc[:, 0:128], Lt_fc[:, 128:256], op=OP.max)
            m23 = wtapool.tile([128, 128], F32, tag="m23")
            nc.vector.tensor_tensor(m23[:], Lt_fc[:, 256:384], Lt_fc[:, 384:512], op=OP.max)
            bm = wtapool.tile([128, 128], F32, tag="bm")
            nc.vector.tensor_tensor(bm[:], m01[:], m23[:], op=OP.max)

            bm4 = bm[:].rearrange("p (a r) -> p a r", a=1).broadcast_to((128, NL, 128))
            msk = wtapool.tile([128, FCW], F32, tag="msk")
            nc.vector.tensor_tensor(
                msk[:].rearrange("p (l r) -> p l r", l=NL),
                Lt_fc.rearrange("p (l r) -> p l r", l=NL),
                bm4,
                op=OP.is_equal,
            )
            wta = wtapool.tile([128, FCW], F32, tag="wta")
            nc.vector.tensor_tensor(wta[:], msk[:], Lt_fc, op=OP.mult)

            # ---- partial cosine sums: dot, nx2, ny2 accumulated per (part,row)
            wv = wta[:].rearrange("p (l sr) -> p l sr", l=NL)
            xv = wv[:, :, 0:64]
            yv = wv[:, :, 64:128]
            prod = prpool.tile([128, 3 * NL * 64], F32, tag="prod")
            for qi, (a, b2) in enumerate(((xv, yv), (xv, xv), (yv, yv))):
                nc.vector.tensor_tensor(
                    prod[:, qi * 256 : (qi + 1) * 256].rearrange(
                        "p (l r) -> p l r", l=NL
                    ),
                    a,
                    b2,
                    op=OP.mult,
                )
            nc.vector.tensor_reduce(
                dtmp[:],
                prod[:].rearrange("p (q l r) -> p q r l", q=3, l=NL),
                axis=AX.X,
                op=OP.add,
            )
            nc.vector.tensor_tensor(acc[:], acc[:], dtmp[:], op=OP.add)

        for fc in range(NFC):
            emit_matmul_stats(fc)
            if fc in (NFC // 2 - 1, NFC - 1):
                g = fc // (NFC // 2)
                emit_group_math(g)
                for f2 in range(g * (NFC // 2), (g + 1) * (NFC // 2)):
                    emit_tanh_wta(f2)

        # ---- cross-partition reduction of the 3 per-row partials
        pf = pf_pool.tile([1, 192], F32, tag="pf")
        nc.tensor.matmul(pf[:], ones[:], acc[:], start=True, stop=True)
        out_sb = persist.tile([1, 192], F32, tag="out_sb")
        nc.scalar.copy(out_sb[:], pf[:])
        nc.sync.dma_start(out_d.ap()[:], out_sb[:])

    return nc


def _host_prep(x, y, W, gamma_x, beta_x, gamma_y, beta_y, mode):
    """Build per-core input maps (numpy only)."""
    import ml_dtypes

    x = np.asarray(x, np.float32)
    y = np.asarray(y, np.float32)
    W = np.asarray(W, np.float32)
    xy = np.concatenate([x, y], axis=0)  # [128, 2048]
    xyt = np.ascontiguousarray(
        xy.T.reshape(KT, 128, 128).transpose(1, 0, 2).reshape(128, K)
    )
    ident = np.eye(128, dtype=np.float32)

    in_maps = []
    j = np.arange(128)
    for c in range(NCORES):
        # permutation: feature (fc, l, j) -> orig 4*(c*1024 + fc*128 + j) + l
        fc_i, l_i, j_i = np.meshgrid(
            np.arange(NFC), np.arange(NL), np.arange(128), indexing="ij"
        )
        perm = (4 * (c * 1024 + fc_i * 128 + j_i) + l_i).reshape(-1)
        Wp = W[perm]  # [4096, 2048]
        wa = np.ascontiguousarray(
            Wp.reshape(NFC, FCW, KT, 128).transpose(0, 3, 2, 1).reshape(NFC, 128, KT * FCW)
        )
        gl = np.zeros((128, 64), np.float32)
        bl = np.zeros((128, 64), np.float32)
        for fc in range(NFC):
            for l in range(NL):
                f_orig = 4 * (c * 1024 + fc * 128 + j) + l
                gl[:, fc * 8 + l * 2 + 0] = gamma_x[f_orig]
                gl[:, fc * 8 + l * 2 + 1] = gamma_y[f_orig]
                bl[:, fc * 8 + l * 2 + 0] = beta_x[f_orig]
                bl[:, fc * 8 + l * 2 + 1] = beta_y[f_orig]
        m = {"gl": gl, "bl": bl, "ident": ident}
        if mode == "bf16x3":
            wa_hi = wa.astype(ml_dtypes.bfloat16)
            wa_lo = (wa - wa_hi.astype(np.float32)).astype(ml_dtypes.bfloat16)
            xyt_hi = xyt.astype(ml_dtypes.bfloat16)
            xyt_lo = (xyt - xyt_hi.astype(np.float32)).astype(ml_dtypes.bfloat16)
            m.update(wa_hi=wa_hi, wa_lo=wa_lo, xyt_hi=xyt_hi, xyt_lo=xyt_lo)
        else:
            m.update(wa=wa, xyt=xyt)
        in_maps.append(m)
    return in_maps


def _finalize(partials):
    """partials: list of 8 arrays [1,192] -> cosine [64] float32."""
    tot = np.sum([np.asarray(p, np.float64).reshape(192) for p in partials], axis=0)
    dot, nx2, ny2 = tot[0:64], tot[64:128], tot[128:192]
    na = np.maximum(np.sqrt(nx2), COS_EPS)
    nb = np.maximum(np.sqrt(ny2), COS_EPS)
    return (dot / (na * nb)).astype(np.float32)


def kernel(x, y, W, b, gamma_x, beta_x, gamma_y, beta_y):
    global LAST_RESULTS
    from concourse.bass_utils import run_bass_kernel_spmd

    mode = MODE
    if mode not in _CACHE:
        nc = _build_nc(mode)
        _split_sync_waits(nc)  # sim chokes on the NoOps; HW path only
        _CACHE[mode] = nc
    nc = _CACHE[mode]

    in_maps = _host_prep(x, y, W, gamma_x, beta_x, gamma_y, beta_y, mode)
    res = run_bass_kernel_spmd(nc, in_maps, core_ids=list(range(NCORES)))
    LAST_RESULTS = res
    return _finalize([r["out"] for r in res.results])


# revision 14
# speedup vs baseline: 1.0187x; 1.0187x over previous
"""Trainium2 Bass kernel for nn_Net_2_78065325572310.

Computes, for x,y [64,2048], W [32768,2048]:
  hx = tanh(BN_train(x@W.T + b)), hy likewise (b cancels in BN, so unused)
  wta = blockwise (4-wide) winner-take-all (keep ==blockmax, zero rest)
  out = cosine(wta(hx), wta(hy)) per row  -> [64]

Sharding: features (W rows) split across 8 cores, 4096 each. Each core
computes per-row partial (dot, ||hx||^2, ||hy||^2) over its features;
host sums partials across cores and finishes the cosine.

Layout on each core:
  - matmul produces h in [row(128=64x|64y) partitions, feat free] via
    lhsT = xyT k-tile [128k, 128col], rhs = W^T strip [128k, 512feat]
  - h tiles are PE-transposed to L layout [block partitions, rows free],
    where host permutation places WTA-block lane l of block (fc,j) at
    feature column l*128+j of chunk fc. BN stats, tanh and WTA are done
    in L layout at full 128-partition width.
"""

import os
import numpy as np

NCORES = 8
BATCH = 64
K = 2048
KT = 16  # k tiles of 128
NFC = 8  # feature chunks per core
FCW = 512  # features per chunk
NL = 4  # WTA block width (lanes)
F_CORE = NFC * FCW  # 4096 features per core
BN_EPS = 1e-5
COS_EPS = 1e-8

# matmul precision mode: "f32" (exact, 4 cyc/row), "f32r" (tf32-ish, 1 cyc/row),
# "bf16x3" (3-pass bf16 hi/lo split, ~fp32 precision, 3 cyc/row)
MODE = os.environ.get("KERNEL_MM_MODE", "bf16x3")

_CACHE = {}
LAST_RESULTS = None


def _split_sync_waits(nc, cap=1):
    """Walrus in this container rejects instructions with >cap sem waits
    ("Too many sync wait commands"); split excess waits onto preceding
    same-engine NoOps."""
    import concourse.mybir as mybir

    n = 0
    for f in nc.m.functions:
        for bb in f.blocks:
            newl = []
            for inst in bb.instructions:
                si = inst.sync_info
                if si is not None and si.on_wait and len(si.on_wait) > cap:
                    waits = list(si.on_wait)
                    keep, extra = waits[:cap], waits[cap:]
                    for k in range(0, len(extra), cap):
                        nop = mybir.InstNoOp(name=f"{inst.name}-wsplit{k}", ins=[], outs=[])
                        nop.engine = inst.engine
                        nop.sync_info = mybir.SyncInfo(
                            on_wait=extra[k : k + cap], on_update=[]
                        )
                        newl.append(nop)
                        n += 1
                    si.on_wait = keep
                newl.append(inst)
            bb.instructions[:] = newl
    return n


def _build_nc(mode):
    import concourse.bass as bass
    import concourse.mybir as mybir
    import concourse.tile as tile
    from contextlib import ExitStack

    F32 = mybir.dt.float32
    BF16 = mybir.dt.bfloat16
    F32R = mybir.dt.float32r
    OP = mybir.AluOpType
    AX = mybir.AxisListType
    AF = mybir.ActivationFunctionType

    nc = bass.Bass(trn_type="TRN2", target_bir_lowering=False, debug=False)

    mm_dt = {"f32": F32, "f32r": F32R, "bf16x3": BF16}[mode]
    if mode == "bf16x3":
        wa_hi = nc.dram_tensor("wa_hi", [NFC, 128, KT * FCW], BF16, kind="ExternalInput")
        wa_lo = nc.dram_tensor("wa_lo", [NFC, 128, KT * FCW], BF16, kind="ExternalInput")
        xyt_hi = nc.dram_tensor("xyt_hi", [128, K], BF16, kind="ExternalInput")
        xyt_lo = nc.dram_tensor("xyt_lo", [128, K], BF16, kind="ExternalInput")
        w_drams = [wa_hi, wa_lo]
    else:
        wa = nc.dram_tensor("wa", [NFC, 128, KT * FCW], mm_dt, kind="ExternalInput")
        xyt = nc.dram_tensor("xyt", [128, K], mm_dt, kind="ExternalInput")
        w_drams = [wa]
    gl_d = nc.dram_tensor("gl", [128, 64], F32, kind="ExternalInput")
    bl_d = nc.dram_tensor("bl", [128, 64], F32, kind="ExternalInput")
    id_d = nc.dram_tensor("ident", [128, 128], F32, kind="ExternalInput")
    out_d = nc.dram_tensor("out", [1, 192], F32, kind="ExternalOutput")

    with tile.TileContext(nc) as tc, ExitStack() as ctx:
        persist = ctx.enter_context(tc.tile_pool(name="persist", bufs=1))
        wpool = ctx.enter_context(tc.tile_pool(name="wpool", bufs=2))
        hpool = ctx.enter_context(tc.tile_pool(name="hpool", bufs=2))
        sqpool = ctx.enter_context(tc.tile_pool(name="sqpool", bufs=2))
        wtapool = ctx.enter_context(tc.tile_pool(name="wtapool", bufs=2))
        prpool = ctx.enter_context(tc.tile_pool(name="prpool", bufs=2))
        ph_pool = ctx.enter_context(tc.tile_pool(name="ph", bufs=3, space="PSUM"))
        pt_pool = ctx.enter_context(tc.tile_pool(name="pt", bufs=4, space="PSUM"))
        pf_pool = ctx.enter_context(tc.tile_pool(name="pf", bufs=1, space="PSUM"))

        # persistent tiles. DMA order matters for startup latency: xy first
        # (gates the first matmul), then the first W chunks are issued inside
        # the fc loop; ident/gl/bl are needed later and follow from there.
        if mode == "bf16x3":
            xy_hi = persist.tile([128, K], BF16, tag="xyhi")
            nc.sync.dma_start(xy_hi[:], xyt_hi.ap()[:])
            xy_lo = persist.tile([128, K], BF16, tag="xylo")
            nc.sync.dma_start(xy_lo[:], xyt_lo.ap()[:])
        else:
            xy_sb = persist.tile([128, K], mm_dt, tag="xy")
            nc.sync.dma_start(xy_sb[:], xyt.ap()[:])
        ident = persist.tile([128, 128], F32, tag="ident")
        gl = persist.tile([128, 64], F32, tag="gl")
        bl = persist.tile([128, 64], F32, tag="bl")

        ones = persist.tile([128, 1], F32, tag="ones")
        nc.vector.memset(ones[:], 1.0)
        magic = persist.tile([128, 64], mybir.dt.uint32, tag="magic")
        nc.vector.memset(magic[:], 0x5F3759DF)

        L_raw = persist.tile([128, NFC * FCW], F32, tag="L_raw")
        L_t = persist.tile([128, NFC * FCW], F32, tag="L_t")
        sums = persist.tile([128, 64], F32, tag="sums")
        sumsq = persist.tile([128, 64], F32, tag="sumsq")
        mean = persist.tile([128, 64], F32, tag="mean")
        e2 = persist.tile([128, 64], F32, tag="e2")
        msq = persist.tile([128, 64], F32, tag="msq")
        veps = persist.tile([128, 64], F32, tag="veps")
        rst = persist.tile([128, 64], F32, tag="rst")
        nt1 = persist.tile([128, 64], F32, tag="nt1")
        nt2 = persist.tile([128, 64], F32, tag="nt2")
        svec = persist.tile([128, 64], F32, tag="svec")
        mts = persist.tile([128, 64], F32, tag="mts")
        tvec = persist.tile([128, 64], F32, tag="tvec")
        acc = persist.tile([128, 192], F32, tag="acc")
        nc.vector.memset(acc[:], 0.0)
        dtmp = persist.tile([128, 192], F32, tag="dtmp")

        NCH = 4  # W strip DMA chunks (separate tiles so matmuls start early)
        KTC = KT // NCH  # k-tiles per chunk

        def emit_matmul_stats(fc):
            # ---- load W strip chunks and matmul into psum h [128 rows, 512f]
            ph = ph_pool.tile([128, FCW], F32, tag="ph")
            CW = KTC * FCW  # free width of one chunk
            if mode == "bf16x3":
                w_hi, w_lo = [], []
                for ch in range(NCH):
                    th = wpool.tile([128, CW], BF16, tag=f"whi{ch}")
                    nc.sync.dma_start(th[:], wa_hi.ap()[fc][:, ch * CW : (ch + 1) * CW])
                    tl = wpool.tile([128, CW], BF16, tag=f"wlo{ch}")
                    nc.sync.dma_start(tl[:], wa_lo.ap()[fc][:, ch * CW : (ch + 1) * CW])
                    w_hi.append(th)
                    w_lo.append(tl)
                n_mm = 3 * KT
                i = 0
                for kt in range(KT):
                    ch, kl = kt // KTC, kt % KTC
                    pairs = [
                        (xy_hi, w_hi[ch]),
                        (xy_lo, w_hi[ch]),
                        (xy_hi, w_lo[ch]),
                    ]
                    for lhs_t, rhs_t in pairs:
                        nc.tensor.matmul(
                            ph[:],
                            lhs_t[:, kt * 128 : (kt + 1) * 128],
                            rhs_t[:, kl * FCW : (kl + 1) * FCW],
                            start=(i == 0),
                            stop=(i == n_mm - 1),
                        )
                        i += 1
            else:
                w_ch = []
                for ch in range(NCH):
                    t = wpool.tile([128, CW], mm_dt, tag=f"w{ch}")
                    nc.sync.dma_start(t[:], w_drams[0].ap()[fc][:, ch * CW : (ch + 1) * CW])
                    w_ch.append(t)
                for kt in range(KT):
                    ch, kl = kt // KTC, kt % KTC
                    nc.tensor.matmul(
                        ph[:],
                        xy_sb[:, kt * 128 : (kt + 1) * 128],
                        w_ch[ch][:, kl * FCW : (kl + 1) * FCW],
                        start=(kt == 0),
                        stop=(kt == KT - 1),
                    )
            if fc == 0:
                nc.sync.dma_start(ident[:], id_d.ap()[:])
            elif fc == 1:
                nc.sync.dma_start(gl[:], gl_d.ap()[:])
                nc.sync.dma_start(bl[:], bl_d.ap()[:])

            # ---- copy h to SBUF (DVE), transpose 128x128 blocks to L layout
            h_sb = hpool.tile([128, FCW], F32, tag="h")
            nc.vector.tensor_copy(h_sb[:], ph[:])
            for l in range(NL):
                pt = pt_pool.tile([128, 128], F32, tag="pt")
                nc.tensor.transpose(
                    pt[:], h_sb[:, l * 128 : (l + 1) * 128], ident[:]
                )
                nc.scalar.copy(L_raw[:, fc * FCW + l * 128 : fc * FCW + (l + 1) * 128], pt[:])

            Lr_fc = L_raw[:, fc * FCW : (fc + 1) * FCW]
            # ---- BN stats: per-feature sums and sum of squares over 64 rows
            S = slice(fc * 8, fc * 8 + 8)
            nc.vector.tensor_reduce(
                sums[:, S],
                Lr_fc.rearrange("p (l s r) -> p l s r", l=NL, s=2),
                axis=AX.X,
                op=OP.add,
            )
            sq = sqpool.tile([128, FCW], F32, tag="sq")
            nc.scalar.square(sq[:], Lr_fc)
            nc.vector.tensor_reduce(
                sumsq[:, S],
                sq[:].rearrange("p (l s r) -> p l s r", l=NL, s=2),
                axis=AX.X,
                op=OP.add,
            )
        GS = 2  # fc chunks per stats-math batch

        def emit_group_math(g):
            # stats math batched over GS fc chunks: mean, var+eps,
            # rstd via fast-inverse-sqrt + 3 Newton steps, s = rstd*gamma,
            # t = beta - mean*s
            S = slice(g * 8 * GS, (g + 1) * 8 * GS)
            nc.vector.tensor_scalar(mean[:, S], sums[:, S], 1.0 / BATCH, None, op0=OP.mult)
            nc.vector.tensor_scalar(
                e2[:, S], sumsq[:, S], 1.0 / BATCH, BN_EPS, op0=OP.mult, op1=OP.add
            )
            nc.vector.tensor_tensor(msq[:, S], mean[:, S], mean[:, S], op=OP.mult)
            nc.vector.tensor_tensor(veps[:, S], e2[:, S], msq[:, S], op=OP.subtract)
            vu = veps[:, S].bitcast(mybir.dt.uint32)
            ru = rst[:, S].bitcast(mybir.dt.uint32)
            nc.vector.tensor_scalar(ru, vu, 1, None, op0=OP.logical_shift_right)
            nc.vector.tensor_tensor(ru, magic[:, S], ru, op=OP.subtract)
            for _ in range(3):
                nc.vector.tensor_tensor(nt1[:, S], rst[:, S], rst[:, S], op=OP.mult)
                nc.vector.tensor_tensor(nt2[:, S], veps[:, S], nt1[:, S], op=OP.mult)
                nc.vector.tensor_scalar(
                    nt2[:, S], nt2[:, S], -0.5, 1.5, op0=OP.mult, op1=OP.add
                )
                nc.vector.tensor_tensor(rst[:, S], rst[:, S], nt2[:, S], op=OP.mult)
            nc.vector.tensor_tensor(svec[:, S], rst[:, S], gl[:, S], op=OP.mult)
            nc.vector.tensor_tensor(mts[:, S], mean[:, S], svec[:, S], op=OP.mult)
            nc.vector.tensor_tensor(tvec[:, S], bl[:, S], mts[:, S], op=OP.subtract)

        def emit_tanh_wta(fc):
            # ---- normalize + tanh (fused on ACT, per (lane, x|y) column)
            for l in range(NL):
                for s in range(2):
                    c = fc * 8 + l * 2 + s
                    off = fc * FCW + l * 128 + s * 64
                    nc.scalar.activation(
                        L_t[:, off : off + 64],
                        L_raw[:, off : off + 64],
                        AF.Tanh,
                        bias=tvec[:, c : c + 1],
                        scale=svec[:, c : c + 1],
                    )

            # ---- WTA: blockmax over 4 lanes, keep ==max
            Lt_fc = L_t[:, fc * FCW : (fc + 1) * FCW]
            m01 = wtapool.tile([128, 128], F32, tag="m01")
            nc.vector.tensor_tensor(m01[:], Lt_fc[:, 0:128], Lt_fc[:, 128:256], op=OP.max)
            m23 = wtapool.tile([128, 128], F32, tag="m23")
            nc.vector.tensor_tensor(m23[:], Lt_fc[:, 256:384], Lt_fc[:, 384:512], op=OP.max)
            bm = wtapool.tile([128, 128], F32, tag="bm")
            nc.vector.tensor_tensor(bm[:], m01[:], m23[:], op=OP.max)

            bm4 = bm[:].rearrange("p (a r) -> p a r", a=1).broadcast_to((128, NL, 128))
            msk = wtapool.tile([128, FCW], F32, tag="msk")
            nc.vector.tensor_tensor(
                msk[:].rearrange("p (l r) -> p l r", l=NL),
                Lt_fc.rearrange("p (l r) -> p l r", l=NL),
                bm4,
                op=OP.is_equal,
            )
            wta = wtapool.tile([128, FCW], F32, tag="wta")
            nc.vector.tensor_tensor(wta[:], msk[:], Lt_fc, op=OP.mult)

            # ---- partial cosine sums: dot, nx2, ny2 accumulated per (part,row)
            wv = wta[:].rearrange("p (l sr) -> p l sr", l=NL)
            xv = wv[:, :, 0:64]
            yv = wv[:, :, 64:128]
            prod = prpool.tile([128, 3 * NL * 64], F32, tag="prod")
            for qi, (a, b2) in enumerate(((xv, yv), (xv, xv), (yv, yv))):
                nc.vector.tensor_tensor(
                    prod[:, qi * 256 : (qi + 1) * 256].rearrange(
                        "p (l r) -> p l r", l=NL
                    ),
                    a,
                    b2,
                    op=OP.mult,
                )
            nc.vector.tensor_reduce(
                dtmp[:],
                prod[:].rearrange("p (q l r) -> p q r l", q=3, l=NL),
                axis=AX.X,
                op=OP.add,
            )
            nc.vector.tensor_tensor(acc[:], acc[:], dtmp[:], op=OP.add)

        for fc in range(NFC):
            emit_matmul_stats(fc)
            if fc % GS == GS - 1:
                g = fc // GS
                emit_group_math(g)
                for f2 in range(g * GS, (g + 1) * GS):
                    emit_tanh_wta(f2)

        # ---- cross-partition reduction of the 3 per-row partials
        pf = pf_pool.tile([1, 192], F32, tag="pf")
        nc.tensor.matmul(pf[:], ones[:], acc[:], start=True, stop=True)
        out_sb = persist.tile([1, 192], F32, tag="out_sb")
        nc.scalar.copy(out_sb[:], pf[:])
        nc.sync.dma_start(out_d.ap()[:], out_sb[:])

    return nc


def _host_prep(x, y, W, gamma_x, beta_x, gamma_y, beta_y, mode):
    """Build per-core input maps (numpy only)."""
    import ml_dtypes

    x = np.asarray(x, np.float32)
    y = np.asarray(y, np.float32)
    W = np.asarray(W, np.float32)
    xy = np.concatenate([x, y], axis=0)  # [128, 2048]
    xyt = np.ascontiguousarray(
        xy.T.reshape(KT, 128, 128).transpose(1, 0, 2).reshape(128, K)
    )
    ident = np.eye(128, dtype=np.float32)

    in_maps = []
    j = np.arange(128)
    for c in range(NCORES):
        # permutation: feature (fc, l, j) -> orig 4*(c*1024 + fc*128 + j) + l
        fc_i, l_i, j_i = np.meshgrid(
            np.arange(NFC), np.arange(NL), np.arange(128), indexing="ij"
        )
        perm = (4 * (c * 1024 + fc_i * 128 + j_i) + l_i).reshape(-1)
        Wp = W[perm]  # [4096, 2048]
        wa = np.ascontiguousarray(
            Wp.reshape(NFC, FCW, KT, 128).transpose(0, 3, 2, 1).reshape(NFC, 128, KT * FCW)
        )
        gl = np.zeros((128, 64), np.float32)
        bl = np.zeros((128, 64), np.float32)
        for fc in range(NFC):
            for l in range(NL):
                f_orig = 4 * (c * 1024 + fc * 128 + j) + l
                gl[:, fc * 8 + l * 2 + 0] = gamma_x[f_orig]
                gl[:, fc * 8 + l * 2 + 1] = gamma_y[f_orig]
                bl[:, fc * 8 + l * 2 + 0] = beta_x[f_orig]
                bl[:, fc * 8 + l * 2 + 1] = beta_y[f_orig]
        m = {"gl": gl, "bl": bl, "ident": ident}
        if mode == "bf16x3":
            wa_hi = wa.astype(ml_dtypes.bfloat16)
            wa_lo = (wa - wa_hi.astype(np.float32)).astype(ml_dtypes.bfloat16)
            xyt_hi = xyt.astype(ml_dtypes.bfloat16)
            xyt_lo = (xyt - xyt_hi.astype(np.float32)).astype(ml_dtypes.bfloat16)
            m.update(wa_hi=wa_hi, wa_lo=wa_lo, xyt_hi=xyt_hi, xyt_lo=xyt_lo)
        else:
            m.update(wa=wa, xyt=xyt)
        in_maps.append(m)
    return in_maps


def _finalize(partials):
    """partials: list of 8 arrays [1,192] -> cosine [64] float32."""
    tot = np.sum([np.asarray(p, np.float64).reshape(192) for p in partials], axis=0)
    dot, nx2, ny2 = tot[0:64], tot[64:128], tot[128:192]
    na = np.maximum(np.sqrt(nx2), COS_EPS)
    nb = np.maximum(np.sqrt(ny2), COS_EPS)
    return (dot / (na * nb)).astype(np.float32)


def kernel(x, y, W, b, gamma_x, beta_x, gamma_y, beta_y):
    global LAST_RESULTS
    from concourse.bass_utils import run_bass_kernel_spmd

    mode = MODE
    if mode not in _CACHE:
        nc = _build_nc(mode)
        _split_sync_waits(nc)  # sim chokes on the NoOps; HW path only
        _CACHE[mode] = nc
    nc = _CACHE[mode]

    in_maps = _host_prep(x, y, W, gamma_x, beta_x, gamma_y, beta_y, mode)
    res = run_bass_kernel_spmd(nc, in_maps, core_ids=list(range(NCORES)))
    LAST_RESULTS = res
    return _finalize([r["out"] for r in res.results])


# revision 15
# speedup vs baseline: 1.1237x; 1.1030x over previous
"""Trainium2 Bass kernel for nn_Net_2_78065325572310.

Computes, for x,y [64,2048], W [32768,2048]:
  hx = tanh(BN_train(x@W.T + b)), hy likewise (b cancels in BN, so unused)
  wta = blockwise (4-wide) winner-take-all (keep ==blockmax, zero rest)
  out = cosine(wta(hx), wta(hy)) per row  -> [64]

Sharding: features (W rows) split across 8 cores, 4096 each. Each core
computes per-row partial (dot, ||hx||^2, ||hy||^2) over its features;
host sums partials across cores and finishes the cosine.

Layout on each core:
  - matmul produces h in [row(128=64x|64y) partitions, feat free] via
    lhsT = xyT k-tile [128k, 128col], rhs = W^T strip [128k, 512feat]
  - h tiles are PE-transposed to L layout [block partitions, rows free],
    where host permutation places WTA-block lane l of block (fc,j) at
    feature column l*128+j of chunk fc. BN stats, tanh and WTA are done
    in L layout at full 128-partition width.
"""

import os
import numpy as np

NCORES = 8
BATCH = 64
K = 2048
KT = 16  # k tiles of 128
NFC = 8  # feature chunks per core
FCW = 512  # features per chunk
NL = 4  # WTA block width (lanes)
F_CORE = NFC * FCW  # 4096 features per core
BN_EPS = 1e-5
COS_EPS = 1e-8

# matmul precision mode: "f32" (exact, 4 cyc/row), "f32r" (tf32-ish, 1 cyc/row),
# "bf16x3" (3-pass bf16 hi/lo split, ~fp32 precision, 3 cyc/row)
MODE = os.environ.get("KERNEL_MM_MODE", "bf16x3")

_CACHE = {}
LAST_RESULTS = None


def _split_sync_waits(nc, cap=1):
    """Walrus in this container rejects instructions with >cap sem waits
    ("Too many sync wait commands"); split excess waits onto preceding
    same-engine NoOps."""
    import concourse.mybir as mybir

    n = 0
    for f in nc.m.functions:
        for bb in f.blocks:
            newl = []
            for inst in bb.instructions:
                si = inst.sync_info
                if si is not None and si.on_wait and len(si.on_wait) > cap:
                    waits = list(si.on_wait)
                    keep, extra = waits[:cap], waits[cap:]
                    for k in range(0, len(extra), cap):
                        nop = mybir.InstNoOp(name=f"{inst.name}-wsplit{k}", ins=[], outs=[])
                        nop.engine = inst.engine
                        nop.sync_info = mybir.SyncInfo(
                            on_wait=extra[k : k + cap], on_update=[]
                        )
                        newl.append(nop)
                        n += 1
                    si.on_wait = keep
                newl.append(inst)
            bb.instructions[:] = newl
    return n


def _build_nc(mode):
    import concourse.bass as bass
    import concourse.mybir as mybir
    import concourse.tile as tile
    from contextlib import ExitStack

    F32 = mybir.dt.float32
    BF16 = mybir.dt.bfloat16
    F32R = mybir.dt.float32r
    OP = mybir.AluOpType
    AX = mybir.AxisListType
    AF = mybir.ActivationFunctionType

    nc = bass.Bass(trn_type="TRN2", target_bir_lowering=False, debug=False)

    mm_dt = {"f32": F32, "f32r": F32R, "bf16x3": BF16}[mode]
    if mode == "bf16x3":
        wa_hi = nc.dram_tensor("wa_hi", [NFC, 128, KT * FCW], BF16, kind="ExternalInput")
        wa_lo = nc.dram_tensor("wa_lo", [NFC, 128, KT * FCW], BF16, kind="ExternalInput")
        xyt_hi = nc.dram_tensor("xyt_hi", [128, K], BF16, kind="ExternalInput")
        xyt_lo = nc.dram_tensor("xyt_lo", [128, K], BF16, kind="ExternalInput")
        w_drams = [wa_hi, wa_lo]
    else:
        wa = nc.dram_tensor("wa", [NFC, 128, KT * FCW], mm_dt, kind="ExternalInput")
        xyt = nc.dram_tensor("xyt", [128, K], mm_dt, kind="ExternalInput")
        w_drams = [wa]
    gl_d = nc.dram_tensor("gl", [128, 64], F32, kind="ExternalInput")
    bl_d = nc.dram_tensor("bl", [128, 64], F32, kind="ExternalInput")
    id_d = nc.dram_tensor("ident", [128, 128], F32, kind="ExternalInput")
    out_d = nc.dram_tensor("out", [1, 192], F32, kind="ExternalOutput")

    with tile.TileContext(nc) as tc, ExitStack() as ctx:
        persist = ctx.enter_context(tc.tile_pool(name="persist", bufs=1))
        wpool = ctx.enter_context(tc.tile_pool(name="wpool", bufs=2))
        hpool = ctx.enter_context(tc.tile_pool(name="hpool", bufs=2))
        sqpool = ctx.enter_context(tc.tile_pool(name="sqpool", bufs=2))
        wtapool = ctx.enter_context(tc.tile_pool(name="wtapool", bufs=2))
        prpool = ctx.enter_context(tc.tile_pool(name="prpool", bufs=2))
        ph_pool = ctx.enter_context(tc.tile_pool(name="ph", bufs=4, space="PSUM"))
        pt_pool = ctx.enter_context(tc.tile_pool(name="pt", bufs=3, space="PSUM"))
        pf_pool = ctx.enter_context(tc.tile_pool(name="pf", bufs=1, space="PSUM"))

        # persistent tiles. DMA order matters for startup latency: xy first
        # (gates the first matmul), then the first W chunks are issued inside
        # the fc loop; ident/gl/bl are needed later and follow from there.
        if mode == "bf16x3":
            xy_hi = persist.tile([128, K], BF16, tag="xyhi")
            nc.sync.dma_start(xy_hi[:], xyt_hi.ap()[:])
            xy_lo = persist.tile([128, K], BF16, tag="xylo")
            nc.sync.dma_start(xy_lo[:], xyt_lo.ap()[:])
        else:
            xy_sb = persist.tile([128, K], mm_dt, tag="xy")
            nc.sync.dma_start(xy_sb[:], xyt.ap()[:])
        ident = persist.tile([128, 128], F32, tag="ident")
        gl = persist.tile([128, 64], F32, tag="gl")
        bl = persist.tile([128, 64], F32, tag="bl")

        ones = persist.tile([128, 1], F32, tag="ones")
        nc.vector.memset(ones[:], 1.0)
        magic = persist.tile([128, 64], mybir.dt.uint32, tag="magic")
        nc.vector.memset(magic[:], 0x5F3759DF)

        L_raw = persist.tile([128, NFC * FCW], F32, tag="L_raw")
        L_t = persist.tile([128, NFC * FCW], F32, tag="L_t")
        sums = persist.tile([128, 64], F32, tag="sums")
        sumsq = persist.tile([128, 64], F32, tag="sumsq")
        mean = persist.tile([128, 64], F32, tag="mean")
        e2 = persist.tile([128, 64], F32, tag="e2")
        msq = persist.tile([128, 64], F32, tag="msq")
        veps = persist.tile([128, 64], F32, tag="veps")
        rst = persist.tile([128, 64], F32, tag="rst")
        nt1 = persist.tile([128, 64], F32, tag="nt1")
        nt2 = persist.tile([128, 64], F32, tag="nt2")
        svec = persist.tile([128, 64], F32, tag="svec")
        mts = persist.tile([128, 64], F32, tag="mts")
        tvec = persist.tile([128, 64], F32, tag="tvec")
        acc = persist.tile([128, 192], F32, tag="acc")
        nc.vector.memset(acc[:], 0.0)
        dtmp = persist.tile([128, 192], F32, tag="dtmp")

        NCH = 4  # W strip DMA chunks (separate tiles so matmuls start early)
        KTC = KT // NCH  # k-tiles per chunk

        def emit_matmul_stats(fc):
            # ---- load W strip chunks and matmul into psum h [128 rows, 512f]
            ph = ph_pool.tile([128, FCW], F32, tag="ph")
            CW = KTC * FCW  # free width of one chunk
            if mode == "bf16x3":
                w_hi, w_lo = [], []
                for ch in range(NCH):
                    th = wpool.tile([128, CW], BF16, tag=f"whi{ch}")
                    nc.sync.dma_start(th[:], wa_hi.ap()[fc][:, ch * CW : (ch + 1) * CW])
                    tl = wpool.tile([128, CW], BF16, tag=f"wlo{ch}")
                    nc.sync.dma_start(tl[:], wa_lo.ap()[fc][:, ch * CW : (ch + 1) * CW])
                    w_hi.append(th)
                    w_lo.append(tl)
                n_mm = 3 * KT
                i = 0
                for kt in range(KT):
                    ch, kl = kt // KTC, kt % KTC
                    pairs = [
                        (xy_hi, w_hi[ch]),
                        (xy_lo, w_hi[ch]),
                        (xy_hi, w_lo[ch]),
                    ]
                    for lhs_t, rhs_t in pairs:
                        nc.tensor.matmul(
                            ph[:],
                            lhs_t[:, kt * 128 : (kt + 1) * 128],
                            rhs_t[:, kl * FCW : (kl + 1) * FCW],
                            start=(i == 0),
                            stop=(i == n_mm - 1),
                        )
                        i += 1
            else:
                w_ch = []
                for ch in range(NCH):
                    t = wpool.tile([128, CW], mm_dt, tag=f"w{ch}")
                    nc.sync.dma_start(t[:], w_drams[0].ap()[fc][:, ch * CW : (ch + 1) * CW])
                    w_ch.append(t)
                for kt in range(KT):
                    ch, kl = kt // KTC, kt % KTC
                    nc.tensor.matmul(
                        ph[:],
                        xy_sb[:, kt * 128 : (kt + 1) * 128],
                        w_ch[ch][:, kl * FCW : (kl + 1) * FCW],
                        start=(kt == 0),
                        stop=(kt == KT - 1),
                    )
            if fc == 0:
                nc.sync.dma_start(ident[:], id_d.ap()[:])
            elif fc == 1:
                nc.sync.dma_start(gl[:], gl_d.ap()[:])
                nc.sync.dma_start(bl[:], bl_d.ap()[:])

            # ---- copy h to SBUF (DVE), transpose 128x128 blocks to L layout
            h_sb = hpool.tile([128, FCW], F32, tag="h")
            nc.scalar.copy(h_sb[:], ph[:])
            for l in range(NL):
                pt = pt_pool.tile([128, 128], F32, tag="pt")
                nc.tensor.transpose(
                    pt[:], h_sb[:, l * 128 : (l + 1) * 128], ident[:]
                )
                nc.scalar.copy(L_raw[:, fc * FCW + l * 128 : fc * FCW + (l + 1) * 128], pt[:])

            Lr_fc = L_raw[:, fc * FCW : (fc + 1) * FCW]
            # ---- BN stats: per-feature sums and sum of squares over 64 rows
            S = slice(fc * 8, fc * 8 + 8)
            nc.vector.tensor_reduce(
                sums[:, S],
                Lr_fc.rearrange("p (l s r) -> p l s r", l=NL, s=2),
                axis=AX.X,
                op=OP.add,
            )
            sq = sqpool.tile([128, FCW], F32, tag="sq")
            nc.scalar.square(sq[:], Lr_fc)
            nc.vector.tensor_reduce(
                sumsq[:, S],
                sq[:].rearrange("p (l s r) -> p l s r", l=NL, s=2),
                axis=AX.X,
                op=OP.add,
            )
        GS = 2  # fc chunks per stats-math batch

        def emit_group_math(g):
            # stats math batched over GS fc chunks: mean, var+eps,
            # rstd via fast-inverse-sqrt + 3 Newton steps, s = rstd*gamma,
            # t = beta - mean*s
            S = slice(g * 8 * GS, (g + 1) * 8 * GS)
            nc.vector.tensor_scalar(mean[:, S], sums[:, S], 1.0 / BATCH, None, op0=OP.mult)
            nc.vector.tensor_scalar(
                e2[:, S], sumsq[:, S], 1.0 / BATCH, BN_EPS, op0=OP.mult, op1=OP.add
            )
            nc.vector.tensor_tensor(msq[:, S], mean[:, S], mean[:, S], op=OP.mult)
            nc.vector.tensor_tensor(veps[:, S], e2[:, S], msq[:, S], op=OP.subtract)
            vu = veps[:, S].bitcast(mybir.dt.uint32)
            ru = rst[:, S].bitcast(mybir.dt.uint32)
            nc.vector.tensor_scalar(ru, vu, 1, None, op0=OP.logical_shift_right)
            nc.vector.tensor_tensor(ru, magic[:, S], ru, op=OP.subtract)
            for _ in range(2):
                nc.vector.tensor_tensor(nt1[:, S], rst[:, S], rst[:, S], op=OP.mult)
                nc.vector.tensor_tensor(nt2[:, S], veps[:, S], nt1[:, S], op=OP.mult)
                nc.vector.tensor_scalar(
                    nt2[:, S], nt2[:, S], -0.5, 1.5, op0=OP.mult, op1=OP.add
                )
                nc.vector.tensor_tensor(rst[:, S], rst[:, S], nt2[:, S], op=OP.mult)
            nc.vector.tensor_tensor(svec[:, S], rst[:, S], gl[:, S], op=OP.mult)
            nc.vector.tensor_tensor(mts[:, S], mean[:, S], svec[:, S], op=OP.mult)
            nc.vector.tensor_tensor(tvec[:, S], bl[:, S], mts[:, S], op=OP.subtract)

        def emit_tanh_wta(fc):
            # ---- normalize + tanh (fused on ACT, per (lane, x|y) column)
            for l in range(NL):
                for s in range(2):
                    c = fc * 8 + l * 2 + s
                    off = fc * FCW + l * 128 + s * 64
                    nc.scalar.activation(
                        L_t[:, off : off + 64],
                        L_raw[:, off : off + 64],
                        AF.Tanh,
                        bias=tvec[:, c : c + 1],
                        scale=svec[:, c : c + 1],
                    )

            # ---- WTA: blockmax over 4 lanes, keep ==max
            Lt_fc = L_t[:, fc * FCW : (fc + 1) * FCW]
            m01 = wtapool.tile([128, 128], F32, tag="m01")
            nc.vector.tensor_tensor(m01[:], Lt_fc[:, 0:128], Lt_fc[:, 128:256], op=OP.max)
            m23 = wtapool.tile([128, 128], F32, tag="m23")
            nc.vector.tensor_tensor(m23[:], Lt_fc[:, 256:384], Lt_fc[:, 384:512], op=OP.max)
            bm = wtapool.tile([128, 128], F32, tag="bm")
            nc.vector.tensor_tensor(bm[:], m01[:], m23[:], op=OP.max)

            bm4 = bm[:].rearrange("p (a r) -> p a r", a=1).broadcast_to((128, NL, 128))
            msk = wtapool.tile([128, FCW], F32, tag="msk")
            nc.vector.tensor_tensor(
                msk[:].rearrange("p (l r) -> p l r", l=NL),
                Lt_fc.rearrange("p (l r) -> p l r", l=NL),
                bm4,
                op=OP.is_equal,
            )
            wta = wtapool.tile([128, FCW], F32, tag="wta")
            nc.vector.tensor_tensor(wta[:], msk[:], Lt_fc, op=OP.mult)

            # ---- partial cosine sums: dot, nx2, ny2 accumulated per (part,row)
            wv = wta[:].rearrange("p (l sr) -> p l sr", l=NL)
            xv = wv[:, :, 0:64]
            yv = wv[:, :, 64:128]
            prod = prpool.tile([128, 3 * NL * 64], F32, tag="prod")
            for qi, (a, b2) in enumerate(((xv, yv), (xv, xv), (yv, yv))):
                nc.vector.tensor_tensor(
                    prod[:, qi * 256 : (qi + 1) * 256].rearrange(
                        "p (l r) -> p l r", l=NL
                    ),
                    a,
                    b2,
                    op=OP.mult,
                )
            nc.vector.tensor_reduce(
                dtmp[:],
                prod[:].rearrange("p (q l r) -> p q r l", q=3, l=NL),
                axis=AX.X,
                op=OP.add,
            )
            nc.vector.tensor_tensor(acc[:], acc[:], dtmp[:], op=OP.add)

        for fc in range(NFC):
            emit_matmul_stats(fc)
            if fc % GS == GS - 1:
                g = fc // GS
                emit_group_math(g)
                for f2 in range(g * GS, (g + 1) * GS):
                    emit_tanh_wta(f2)

        # ---- cross-partition reduction of the 3 per-row partials
        pf = pf_pool.tile([1, 192], F32, tag="pf")
        nc.tensor.matmul(pf[:], ones[:], acc[:], start=True, stop=True)
        out_sb = persist.tile([1, 192], F32, tag="out_sb")
        nc.scalar.copy(out_sb[:], pf[:])
        nc.sync.dma_start(out_d.ap()[:], out_sb[:])

    return nc


def _host_prep(x, y, W, gamma_x, beta_x, gamma_y, beta_y, mode):
    """Build per-core input maps (numpy only)."""
    import ml_dtypes

    x = np.asarray(x, np.float32)
    y = np.asarray(y, np.float32)
    W = np.asarray(W, np.float32)
    xy = np.concatenate([x, y], axis=0)  # [128, 2048]
    xyt = np.ascontiguousarray(
        xy.T.reshape(KT, 128, 128).transpose(1, 0, 2).reshape(128, K)
    )
    ident = np.eye(128, dtype=np.float32)

    in_maps = []
    j = np.arange(128)
    for c in range(NCORES):
        # permutation: feature (fc, l, j) -> orig 4*(c*1024 + fc*128 + j) + l
        fc_i, l_i, j_i = np.meshgrid(
            np.arange(NFC), np.arange(NL), np.arange(128), indexing="ij"
        )
        perm = (4 * (c * 1024 + fc_i * 128 + j_i) + l_i).reshape(-1)
        Wp = W[perm]  # [4096, 2048]
        wa = np.ascontiguousarray(
            Wp.reshape(NFC, FCW, KT, 128).transpose(0, 3, 2, 1).reshape(NFC, 128, KT * FCW)
        )
        gl = np.zeros((128, 64), np.float32)
        bl = np.zeros((128, 64), np.float32)
        for fc in range(NFC):
            for l in range(NL):
                f_orig = 4 * (c * 1024 + fc * 128 + j) + l
                gl[:, fc * 8 + l * 2 + 0] = gamma_x[f_orig]
                gl[:, fc * 8 + l * 2 + 1] = gamma_y[f_orig]
                bl[:, fc * 8 + l * 2 + 0] = beta_x[f_orig]
                bl[:, fc * 8 + l * 2 + 1] = beta_y[f_orig]
        m = {"gl": gl, "bl": bl, "ident": ident}
        if mode == "bf16x3":
            wa_hi = wa.astype(ml_dtypes.bfloat16)
            wa_lo = (wa - wa_hi.astype(np.float32)).astype(ml_dtypes.bfloat16)
            xyt_hi = xyt.astype(ml_dtypes.bfloat16)
            xyt_lo = (xyt - xyt_hi.astype(np.float32)).astype(ml_dtypes.bfloat16)
            m.update(wa_hi=wa_hi, wa_lo=wa_lo, xyt_hi=xyt_hi, xyt_lo=xyt_lo)
        else:
            m.update(wa=wa, xyt=xyt)
        in_maps.append(m)
    return in_maps


def _finalize(partials):
    """partials: list of 8 arrays [1,192] -> cosine [64] float32."""
    tot = np.sum([np.asarray(p, np.float64).reshape(192) for p in partials], axis=0)
    dot, nx2, ny2 = tot[0:64], tot[64:128], tot[128:192]
    na = np.maximum(np.sqrt(nx2), COS_EPS)
    nb = np.maximum(np.sqrt(ny2), COS_EPS)
    return (dot / (na * nb)).astype(np.float32)


def kernel(x, y, W, b, gamma_x, beta_x, gamma_y, beta_y):
    global LAST_RESULTS
    from concourse.bass_utils import run_bass_kernel_spmd

    mode = MODE
    if mode not in _CACHE:
        nc = _build_nc(mode)
        _split_sync_waits(nc)  # sim chokes on the NoOps; HW path only
        _CACHE[mode] = nc
    nc = _CACHE[mode]

    in_maps = _host_prep(x, y, W, gamma_x, beta_x, gamma_y, beta_y, mode)
    res = run_bass_kernel_spmd(nc, in_maps, core_ids=list(range(NCORES)))
    LAST_RESULTS = res
    return _finalize([r["out"] for r in res.results])
